# revision 29
# baseline (speedup 1.0000x reference)
"""Trainium2 Bass kernel for nn_ComplexityAttention (mu-gated GQA attention).

Distribution: 8 NeuronCores = 2 batches x 4 kv-groups. Core c handles
batch b=c//4 and kv-group g=c%4 (4 q-heads + 1 kv-head). No collectives:
each core emits a partial output [N, D] (its head-group's contribution
through the row-sharded wo), and the host sums the 4 partials per batch.

Device pipeline per core:
  A) QKV projection (bf16 matmuls, f32 PSUM accum over 4096-dim concat
     [x, mu_prev] features) -> per-head RMSNorm (ACT square+accum,
     sqrt, DVE reciprocal) -> RoPE (DVE, tables with norm-weights folded
     in; even/odd dims pre-permuted on host so RoPE is contiguous)
     -> PE transpose of Q,K into [hd, token] layout.
  B) Attention per (head, query-block of 512): S^T = K @ Q^T on PE,
     exp on ACT (no max-subtraction; scores are bounded), P@V and
     ones-reduction (softmax denominator) accumulated on PE,
     reciprocal + normalize on DVE, gpsimd partition-broadcast.
  C) Output projection out^T_h @ wo_h^T accumulated over 4 heads.

Host does all layout work: transposes, tiling, bf16 conversion,
even/odd permutation of wq/wk rows, folding q/k norm weights into
cos/sin tables, and the final sum over kv-groups.
"""

import os
import sys
import types
from contextlib import ExitStack

import numpy as np

for _p in ("/opt/trn_rl_repo", "/root/.axon_site/_ro/trn_rl_repo"):
    if os.path.isdir(_p) and _p not in sys.path:
        sys.path.append(_p)

import ml_dtypes  # noqa: E402
import concourse.bass as bass  # noqa: E402
import concourse.tile as tile  # noqa: E402
from concourse import bacc, mybir  # noqa: E402
from concourse.bass_utils import run_bass_kernel_spmd  # noqa: E402
from concourse.masks import make_identity  # noqa: E402

BF16 = mybir.dt.bfloat16
F32 = mybir.dt.float32
AF = mybir.ActivationFunctionType

B, N, D = 2, 2048, 2048
H, KVH = 16, 4
HD = 128                 # head dim
REP = H // KVH           # q heads per kv group (= per core)
QDIM = REP * HD          # 512 q dims per core
F2 = 2 * D               # 4096 concatenated feature dim
KT = F2 // 128           # 32 contraction tiles
MT = N // 128            # 16 token tiles
NQB = 4                  # query blocks of 512
EPS = 1e-6
SCALE = float(HD) ** -0.5
NCORES = 8

_nc_cache = None


def _kernel_body(tc, xmu, wq, wkv, wo, tabs, out):
    nc = tc.nc
    with ExitStack() as ctx:
        singles = ctx.enter_context(tc.tile_pool(name="singles", bufs=1))
        xpool = ctx.enter_context(tc.tile_pool(name="xpool", bufs=3))
        tpool = ctx.enter_context(tc.tile_pool(name="tpool", bufs=2))
        wpool = ctx.enter_context(tc.tile_pool(name="wpool", bufs=3))
        ppool = ctx.enter_context(tc.tile_pool(name="ppool", bufs=8))
        apool = ctx.enter_context(tc.tile_pool(name="apool", bufs=2))
        opool = ctx.enter_context(tc.tile_pool(name="opool", bufs=3))
        dpool = ctx.enter_context(tc.tile_pool(name="dpool", bufs=4))
        psum = ctx.enter_context(tc.tile_pool(name="psum", bufs=2, space="PSUM"))

        def load_m(m):
            xt = xpool.tile([128, KT, 128], BF16, tag="xt", name="xt")
            nc.sync.dma_start(xt, xmu[m])
            qtab = tpool.tile([128, 16, 64], BF16, tag="qtab", name="qtab")
            nc.sync.dma_start(qtab, tabs[m][:, 0:16])
            ktab = tpool.tile([128, 4, 64], BF16, tag="ktab", name="ktab")
            nc.sync.dma_start(ktab, tabs[m][:, 16:20])
            return xt, qtab, ktab

        # First iteration's x before the big weight DMAs so the PE can
        # start as soon as chunk 0 of wqkv lands.
        xt0 = xpool.tile([128, KT, 128], BF16, tag="xt", name="xt")
        nc.sync.dma_start(xt0, xmu[0])
        # weights ride the gpsimd (SWDGE) queue so activations/tables on the
        # sync queue aren't stuck behind them
        wq_sb = singles.tile([128, KT, QDIM], BF16)
        k0 = 0
        for ci, csz in enumerate((1, 1, 2, 4, 8, 8, 8)):
            eng = nc.gpsimd if ci % 2 == 0 else nc.scalar
            eng.dma_start(wq_sb[:, k0:k0 + csz], wq[:, k0:k0 + csz])
            k0 += csz
        wkv_sb = singles.tile([128, KT, 256], BF16)
        for ci, k0 in enumerate(range(0, KT, 8)):
            eng = nc.gpsimd if ci % 2 == 0 else nc.scalar
            eng.dma_start(wkv_sb[:, k0:k0 + 8], wkv[:, k0:k0 + 8])
        qtab0 = tpool.tile([128, 16, 64], BF16, tag="qtab", name="qtab")
        nc.sync.dma_start(qtab0, tabs[0][:, 0:16])
        ktab0 = tpool.tile([128, 4, 64], BF16, tag="ktab", name="ktab")
        nc.sync.dma_start(ktab0, tabs[0][:, 16:20])
        loaded0 = (xt0, qtab0, ktab0)
        ident = singles.tile([128, 128], BF16)
        make_identity(nc, ident)
        ones_sb = singles.tile([128, 1], BF16)
        nc.vector.memset(ones_sb, 1.0)
        eps_sb = singles.tile([128, 1], F32)
        nc.vector.memset(eps_sb, EPS)
        wo_sb = singles.tile([128, REP, D], BF16)
        nc.gpsimd.dma_start(wo_sb, wo)
        QT = singles.tile([128, REP, N], BF16)   # q^T per head: [hd, tok]
        KTr = singles.tile([128, N], BF16)       # k^T: [hd, tok]
        Vt = singles.tile([128, MT, HD], BF16)   # v: [tok(128/tile), kt, hd]
        OT = singles.tile([128, REP, N], BF16)   # attn out^T per head

        def transpose_prev(prev):
            # Software-pipelined: runs after the NEXT m's QKV matmuls are
            # emitted, so the PE never waits on the norm/rope chain.
            pm, pqr, pkr = prev
            ptr = psum.tile([128, REP + 1, 128], BF16, tag="ps2", bufs=3, name="ptr")
            for h in range(REP):
                nc.tensor.transpose(ptr[:, h], pqr[:, h], ident)
            nc.tensor.transpose(ptr[:, REP], pkr, ident)
            nc.vector.tensor_copy(QT[:, :, pm * 128:(pm + 1) * 128],
                                  ptr[:, 0:REP])
            nc.vector.tensor_copy(KTr[:, pm * 128:(pm + 1) * 128],
                                  ptr[:, REP])

        # ---- Phase A: QKV + RMSNorm + RoPE + transpose ----
        prev = None
        for m in range(MT):
            xt, qtab, ktab = loaded0 if m == 0 else load_m(m)

            psq = psum.tile([128, QDIM], F32, tag="ps2", bufs=3, name="psq")
            pskv = psum.tile([128, 256], F32, tag="ps", name="pskv")
            for k in range(KT):
                nc.tensor.matmul(psq, xt[:, k], wq_sb[:, k],
                                 start=(k == 0), stop=(k == KT - 1))
            for k in range(KT):
                nc.tensor.matmul(pskv, xt[:, k], wkv_sb[:, k],
                                 start=(k == 0), stop=(k == KT - 1))

            if prev is not None:
                transpose_prev(prev)

            # RMSNorm q (per head) and k
            qn = wpool.tile([128, REP, HD], BF16, tag="qn")
            for h in range(REP):
                sq = wpool.tile([128, HD], F32, tag="sq")
                ssq = wpool.tile([128, 1], F32, tag="ssq")
                nc.scalar.activation(out=sq, in_=psq[:, h * HD:(h + 1) * HD],
                                     func=AF.Square, accum_out=ssq)
                std = wpool.tile([128, 1], F32, tag="std")
                nc.scalar.activation(out=std, in_=ssq, func=AF.Sqrt,
                                     bias=eps_sb, scale=1.0 / HD)
                r = wpool.tile([128, 1], F32, tag="r")
                nc.vector.reciprocal(r, std)
                nc.vector.tensor_scalar_mul(qn[:, h],
                                            psq[:, h * HD:(h + 1) * HD], r)
            kn = wpool.tile([128, HD], BF16, tag="kn")
            sqk = wpool.tile([128, HD], F32, tag="sqk")
            ssqk = wpool.tile([128, 1], F32, tag="ssqk")
            nc.scalar.activation(out=sqk, in_=pskv[:, 0:HD],
                                 func=AF.Square, accum_out=ssqk)
            stdk = wpool.tile([128, 1], F32, tag="stdk")
            nc.scalar.activation(out=stdk, in_=ssqk, func=AF.Sqrt,
                                 bias=eps_sb, scale=1.0 / HD)
            rk = wpool.tile([128, 1], F32, tag="rk")
            nc.vector.reciprocal(rk, stdk)
            nc.vector.tensor_scalar_mul(kn, pskv[:, 0:HD], rk)

            # V: plain copy PSUM -> SBUF (bf16 cast)
            nc.scalar.activation(out=Vt[:, m], in_=pskv[:, HD:256], func=AF.Copy)

            # RoPE q (batched over heads; tables already head-replicated)
            qa = wpool.tile([128, REP, 64], BF16, tag="qa")
            qb2 = wpool.tile([128, REP, 64], BF16, tag="qb2")
            qr = wpool.tile([128, REP, HD], BF16, tag="qr")
            nc.vector.tensor_mul(qa, qn[:, :, 0:64], qtab[:, 0:4])
            nc.vector.tensor_mul(qb2, qn[:, :, 64:128], qtab[:, 4:8])
            nc.vector.tensor_sub(qr[:, :, 0:64], qa, qb2)
            qc = wpool.tile([128, REP, 64], BF16, tag="qc")
            qd = wpool.tile([128, REP, 64], BF16, tag="qd")
            nc.vector.tensor_mul(qc, qn[:, :, 0:64], qtab[:, 8:12])
            nc.vector.tensor_mul(qd, qn[:, :, 64:128], qtab[:, 12:16])
            nc.vector.tensor_add(qr[:, :, 64:128], qc, qd)

            # RoPE k
            ka = wpool.tile([128, 64], BF16, tag="ka")
            kb = wpool.tile([128, 64], BF16, tag="kb")
            kr = wpool.tile([128, HD], BF16, tag="kr")
            nc.vector.tensor_mul(ka, kn[:, 0:64], ktab[:, 0])
            nc.vector.tensor_mul(kb, kn[:, 64:128], ktab[:, 1])
            nc.vector.tensor_sub(kr[:, 0:64], ka, kb)
            kc = wpool.tile([128, 64], BF16, tag="kc")
            kd = wpool.tile([128, 64], BF16, tag="kd")
            nc.vector.tensor_mul(kc, kn[:, 0:64], ktab[:, 2])
            nc.vector.tensor_mul(kd, kn[:, 64:128], ktab[:, 3])
            nc.vector.tensor_add(kr[:, 64:128], kc, kd)

            prev = (m, qr, kr)
        # prev (m=15) transpose is deferred into the first attention group
        # so the PE doesn't stall on the last norm/rope chain.

        def attention(h, qb, defer=None):
            qs = slice(qb * 512, (qb + 1) * 512)
            po = psum.tile([128, 512], F32, tag="ps", name="po")
            pd = psum.tile([1, 512], F32, tag="ps", name="pd")
            NP = MT // 2
            pts = [None] * NP

            dsums = [None] * NP
            dquads = [None] * (NP // 2)
            docts = [None] * (NP // 4)

            def s_exp2(j):
                # kt pair (2j, 2j+1) into one 2-bank PSUM tile; single wide
                # exp on ACT amortizes the per-op overhead below PE pace.
                ps2 = psum.tile([128, 2, 512], F32, tag="ps2", bufs=3,
                                name="ps2")
                k0, k1 = 2 * j, 2 * j + 1
                nc.tensor.matmul(ps2[:, 0], KTr[:, k0 * 128:(k0 + 1) * 128],
                                 QT[:, h, qs], start=True, stop=True)
                if defer is not None and j == NP - 1:
                    transpose_prev(defer)
                nc.tensor.matmul(ps2[:, 1], KTr[:, k1 * 128:(k1 + 1) * 128],
                                 QT[:, h, qs], start=True, stop=True)
                pt = ppool.tile([128, 2, 512], BF16, tag="pt", name="pt")
                nc.scalar.activation(out=pt, in_=ps2, func=AF.Exp,
                                     scale=SCALE)
                pts[j] = pt
                # pair-sum on DVE halves the denominator matmuls on PE
                ds = dpool.tile([128, 512], BF16, tag="ds", name="ds")
                nc.vector.tensor_add(ds, pt[:, 0], pt[:, 1])
                dsums[j] = ds
                if j % 2 == 1:
                    # quad-sum: one denominator matmul per 4 kt
                    dq = dpool.tile([128, 512], BF16, tag="dq", name="dq")
                    nc.vector.tensor_add(dq, dsums[j - 1], ds)
                    dquads[j // 2] = dq
                if j % 4 == 3:
                    do_ = dpool.tile([128, 512], BF16, tag="do_", name="do_")
                    nc.vector.tensor_add(do_, dquads[j // 2 - 1],
                                         dquads[j // 2])
                    docts[j // 4] = do_

            def pvd2(j):
                for s in (0, 1):
                    k = 2 * j + s
                    nc.tensor.matmul(po, Vt[:, k], pts[j][:, s],
                                     start=(k == 0), stop=(k == MT - 1))
                if j % 4 == 3:
                    nc.tensor.matmul(pd, ones_sb, docts[j // 4],
                                     start=(j == 3), stop=(j == NP - 1))

            for j in range(NP):
                s_exp2(j)
                if j >= 1:
                    pvd2(j - 1)
            pvd2(NP - 1)
            rec = apool.tile([1, 512], F32, tag="rec", name="rec")
            nc.vector.reciprocal_approx_fast(out=rec, in_=pd)
            rb = apool.tile([128, 512], F32, tag="rb", name="rb")
            nc.gpsimd.partition_broadcast(rb, rec)
            nc.vector.tensor_mul(OT[:, h, qs], po, rb)

        def out_proj(mm):
            ms = slice(mm * 128, (mm + 1) * 128)
            for obp in range(2):
                # two output blocks share one 2-bank tile: 8 wait-free
                # matmuls (LDWs hide) + a single wide copy
                pp = psum.tile([128, 2, 512], F32, tag="ps2", bufs=3,
                               name="pp")
                for h in range(REP):
                    for s in (0, 1):
                        ob = 2 * obp + s
                        nc.tensor.matmul(pp[:, s], OT[:, h, ms],
                                         wo_sb[:, h, ob * 512:(ob + 1) * 512],
                                         start=(h == 0), stop=(h == REP - 1))
                osb = opool.tile([128, 1024], F32, tag="osb", name="osb")
                nc.scalar.activation(out=osb, in_=pp, func=AF.Copy)
                nc.sync.dma_start(
                    out[ms, obp * 1024:(obp + 1) * 1024], osb)

        # ---- Phase B (qb-outer) with phase C pipelined one qb behind ----
        for qb in range(NQB):
            for h in range(REP):
                attention(h, qb, defer=prev if (qb == 0 and h == 0) else None)
            if qb >= 1:
                for mm in range(4 * (qb - 1), 4 * qb):
                    out_proj(mm)
        for mm in range(4 * (NQB - 1), 4 * NQB):
            out_proj(mm)


def _build_nc():
    nc = bacc.Bacc("TRN2", target_bir_lowering=False, debug=False,
                   num_devices=NCORES)
    xmu = nc.dram_tensor("xmu", [MT, 128, KT, 128], BF16,
                         kind="ExternalInput").ap()
    wq = nc.dram_tensor("wq", [128, KT, QDIM], BF16,
                        kind="ExternalInput").ap()
    wkv = nc.dram_tensor("wkv", [128, KT, 256], BF16,
                         kind="ExternalInput").ap()
    wo = nc.dram_tensor("wo", [128, REP, D], BF16, kind="ExternalInput").ap()
    tabs = nc.dram_tensor("tabs", [MT, 128, 20, 64], BF16,
                          kind="ExternalInput").ap()
    out = nc.dram_tensor("out", [N, D], F32, kind="ExternalOutput").ap()
    with tile.TileContext(nc) as tc:
        _kernel_body(tc, xmu, wq, wkv, wo, tabs, out)
    nc.compile()
    return nc


_PERM = np.concatenate([np.arange(0, HD, 2), np.arange(1, HD, 2)])


def _bf16(a):
    return np.ascontiguousarray(a).astype(ml_dtypes.bfloat16)


def _prep_shared(inputs):
    """Host prep that doesn't depend on the core: tables + per-(b,g) arrays."""
    x = np.asarray(inputs["x"], np.float32)
    mu = np.asarray(inputs["mu_prev"], np.float32)
    cos = np.asarray(inputs["cos"], np.float32)
    sin = np.asarray(inputs["sin"], np.float32)
    qnw = np.asarray(inputs["q_norm_w"], np.float32)
    knw = np.asarray(inputs["k_norm_w"], np.float32)
    wq = np.asarray(inputs["wq"], np.float32)
    wk = np.asarray(inputs["wk"], np.float32)
    wv = np.asarray(inputs["wv"], np.float32)
    mqw = np.asarray(inputs["mu_q_w"], np.float32)
    mkw = np.asarray(inputs["mu_k_w"], np.float32)
    mvw = np.asarray(inputs["mu_v_w"], np.float32)
    wo = np.asarray(inputs["wo"], np.float32)

    # RoPE tables with norm weights folded in (permuted even/odd space):
    # out1 = t1*C1 - t2*S2 ; out2 = t1*S1 + t2*C2
    we, wo_ = qnw[0::2], qnw[1::2]
    qparts = [cos * we, sin * wo_, sin * we, cos * wo_]
    we_k, wo_k = knw[0::2], knw[1::2]
    kparts = [cos * we_k, sin * wo_k, sin * we_k, cos * wo_k]
    tab_list = [qparts[j] for j in range(4) for _ in range(REP)] + kparts
    tabs = np.stack(tab_list, axis=1)                # [N, 20, 64]
    tabs_arr = _bf16(tabs.reshape(MT, 128, 20, 64))

    # Per-batch xmu, pre-tiled [m, f, kt, t]
    xmu_arrs = []
    for b in range(B):
        xm = np.concatenate([x[b], mu[b]], axis=1)   # [N, 4096]
        xm = xm.reshape(MT, 128, KT, 128).transpose(0, 3, 2, 1)
        xmu_arrs.append(_bf16(xm))

    # Per-group weights
    wq_arrs, wkv_arrs, wo_arrs = [], [], []
    for g in range(KVH):
        qs = slice(g * QDIM, (g + 1) * QDIM)
        kvs = slice(g * HD, (g + 1) * HD)
        perm_q = np.concatenate([hh * HD + _PERM for hh in range(REP)])
        Wq = np.concatenate([wq[qs], mqw[qs]], axis=1)[perm_q]   # [512, 4096]
        Wk = np.concatenate([wk[kvs], mkw[kvs]], axis=1)[_PERM]  # [128, 4096]
        Wv = np.concatenate([wv[kvs], mvw[kvs]], axis=1)         # [128, 4096]
        Wg = np.concatenate([Wq, Wk, Wv], axis=0)                # [768, 4096]
        wg_t = Wg.T.reshape(KT, 128, 768).transpose(1, 0, 2)
        wq_arrs.append(_bf16(wg_t[:, :, 0:QDIM]))
        wkv_arrs.append(_bf16(wg_t[:, :, QDIM:768]))
        wo_g = wo[:, g * QDIM:(g + 1) * QDIM].T                  # [512, D]
        wo_arrs.append(_bf16(
            wo_g.reshape(REP, HD, D).transpose(1, 0, 2)))

    in_maps = []
    for c in range(NCORES):
        b, g = divmod(c, KVH)
        in_maps.append({
            "xmu": xmu_arrs[b],
            "wq": wq_arrs[g],
            "wkv": wkv_arrs[g],
            "wo": wo_arrs[g],
            "tabs": tabs_arr,
        })
    return in_maps


def _install_ntff_hook():
    try:
        import antenv.axon_hooks as m
        if m.get_axon_ntff_profile_hook() is not None:
            return True
    except ImportError:
        import antenv
        m = types.ModuleType("antenv.axon_hooks")
        m._hook = None
        m.set_axon_ntff_profile_hook = lambda h: setattr(m, "_hook", h)
        m.get_axon_ntff_profile_hook = lambda: m._hook
        sys.modules["antenv.axon_hooks"] = m
        antenv.axon_hooks = m
    try:
        from trn_agent_boot.trn_boot import _ntff_profile_via_ctypes
        m.set_axon_ntff_profile_hook(
            _ntff_profile_via_ctypes("/opt/axon/libaxon_pjrt.so"))
    except Exception:
        return False
    return m.get_axon_ntff_profile_hook() is not None


def run(inputs, trace=False, tmpdir=None):
    """Returns (output [B,N,D] f32, BassKernelResults)."""
    global _nc_cache
    if trace:
        _install_ntff_hook()
    if _nc_cache is None:
        _nc_cache = _build_nc()
    in_maps = _prep_shared(inputs)
    res = run_bass_kernel_spmd(_nc_cache, in_maps,
                               core_ids=list(range(NCORES)),
                               trace=trace, tmpdir=tmpdir)
    parts = np.stack([np.asarray(res.results[c]["out"], np.float32)
                      for c in range(NCORES)])
    outv = parts.reshape(B, KVH, N, D).sum(axis=1).astype(np.float32)
    return outv, res


def kernel(**inputs):
    outv, _ = run(inputs, trace=False)
    return outv


# revision 30
# speedup vs baseline: 1.0153x; 1.0153x over previous
"""Trainium2 Bass kernel for nn_ComplexityAttention (mu-gated GQA attention).

Distribution: 8 NeuronCores = 2 batches x 4 kv-groups. Core c handles
batch b=c//4 and kv-group g=c%4 (4 q-heads + 1 kv-head). No collectives:
each core emits a partial output [N, D] (its head-group's contribution
through the row-sharded wo), and the host sums the 4 partials per batch.

Device pipeline per core:
  A) QKV projection (bf16 matmuls, f32 PSUM accum over 4096-dim concat
     [x, mu_prev] features) -> per-head RMSNorm (ACT square+accum,
     sqrt, DVE reciprocal) -> RoPE (DVE, tables with norm-weights folded
     in; even/odd dims pre-permuted on host so RoPE is contiguous)
     -> PE transpose of Q,K into [hd, token] layout.
  B) Attention per (head, query-block of 512): S^T = K @ Q^T on PE,
     exp on ACT (no max-subtraction; scores are bounded), P@V and
     ones-reduction (softmax denominator) accumulated on PE,
     reciprocal + normalize on DVE, gpsimd partition-broadcast.
  C) Output projection out^T_h @ wo_h^T accumulated over 4 heads.

Host does all layout work: transposes, tiling, bf16 conversion,
even/odd permutation of wq/wk rows, folding q/k norm weights into
cos/sin tables, and the final sum over kv-groups.
"""

import os
import sys
import types
from contextlib import ExitStack

import numpy as np

for _p in ("/opt/trn_rl_repo", "/root/.axon_site/_ro/trn_rl_repo"):
    if os.path.isdir(_p) and _p not in sys.path:
        sys.path.append(_p)

import ml_dtypes  # noqa: E402
import concourse.bass as bass  # noqa: E402
import concourse.tile as tile  # noqa: E402
from concourse import bacc, mybir  # noqa: E402
from concourse.bass_utils import run_bass_kernel_spmd  # noqa: E402
from concourse.masks import make_identity  # noqa: E402

BF16 = mybir.dt.bfloat16
F32 = mybir.dt.float32
AF = mybir.ActivationFunctionType

B, N, D = 2, 2048, 2048
H, KVH = 16, 4
HD = 128                 # head dim
REP = H // KVH           # q heads per kv group (= per core)
QDIM = REP * HD          # 512 q dims per core
F2 = 2 * D               # 4096 concatenated feature dim
KT = F2 // 128           # 32 contraction tiles
MT = N // 128            # 16 token tiles
NQB = 4                  # query blocks of 512
EPS = 1e-6
SCALE = float(HD) ** -0.5
NCORES = 8

_nc_cache = None


def _kernel_body(tc, xmu, wq, wkv, wo, tabs, out):
    nc = tc.nc
    with ExitStack() as ctx:
        singles = ctx.enter_context(tc.tile_pool(name="singles", bufs=1))
        xpool = ctx.enter_context(tc.tile_pool(name="xpool", bufs=2))
        tpool = ctx.enter_context(tc.tile_pool(name="tpool", bufs=2))
        wpool = ctx.enter_context(tc.tile_pool(name="wpool", bufs=3))
        ppool = ctx.enter_context(tc.tile_pool(name="ppool", bufs=6))
        apool = ctx.enter_context(tc.tile_pool(name="apool", bufs=2))
        opool = ctx.enter_context(tc.tile_pool(name="opool", bufs=3))
        dpool = ctx.enter_context(tc.tile_pool(name="dpool", bufs=4))
        psum = ctx.enter_context(tc.tile_pool(name="psum", bufs=2, space="PSUM"))

        def load_m(m):
            xt = xpool.tile([128, KT, 128], BF16, tag="xt", name="xt")
            nc.sync.dma_start(xt, xmu[m])
            qtab = tpool.tile([128, 16, 64], BF16, tag="qtab", name="qtab")
            nc.sync.dma_start(qtab, tabs[m][:, 0:16])
            ktab = tpool.tile([128, 4, 64], BF16, tag="ktab", name="ktab")
            nc.sync.dma_start(ktab, tabs[m][:, 16:20])
            return xt, qtab, ktab

        # First iteration's x before the big weight DMAs so the PE can
        # start as soon as chunk 0 of wqkv lands.
        xt0 = xpool.tile([128, KT, 128], BF16, tag="xt", name="xt")
        nc.sync.dma_start(xt0, xmu[0])
        # weights ride the gpsimd (SWDGE) queue so activations/tables on the
        # sync queue aren't stuck behind them
        wq_sb = singles.tile([128, KT, QDIM], BF16)
        k0 = 0
        for ci, csz in enumerate((1, 1, 2, 4, 8, 8, 8)):
            eng = nc.gpsimd if ci % 2 == 0 else nc.scalar
            eng.dma_start(wq_sb[:, k0:k0 + csz], wq[:, k0:k0 + csz])
            k0 += csz
        wkv_sb = singles.tile([128, KT, 256], BF16)
        for ci, k0 in enumerate(range(0, KT, 8)):
            eng = nc.gpsimd if ci % 2 == 0 else nc.scalar
            eng.dma_start(wkv_sb[:, k0:k0 + 8], wkv[:, k0:k0 + 8])
        qtab0 = tpool.tile([128, 16, 64], BF16, tag="qtab", name="qtab")
        nc.sync.dma_start(qtab0, tabs[0][:, 0:16])
        ktab0 = tpool.tile([128, 4, 64], BF16, tag="ktab", name="ktab")
        nc.sync.dma_start(ktab0, tabs[0][:, 16:20])
        loaded0 = (xt0, qtab0, ktab0)
        ident = singles.tile([128, 128], BF16)
        make_identity(nc, ident)
        ones_sb = singles.tile([128, 1], BF16)
        nc.vector.memset(ones_sb, 1.0)
        eps_sb = singles.tile([128, 1], F32)
        nc.vector.memset(eps_sb, EPS)
        wo_sb = singles.tile([128, REP, D], BF16)
        nc.gpsimd.dma_start(wo_sb, wo)
        QT = singles.tile([128, REP, N], BF16)   # q^T per head: [hd, tok]
        KTr = singles.tile([128, N], BF16)       # k^T: [hd, tok]
        Vt = singles.tile([128, MT, HD], BF16)   # v: [tok(128/tile), kt, hd]
        OT = singles.tile([128, REP, N], BF16)   # attn out^T per head

        def transpose_prev(prev):
            # Software-pipelined: runs after the NEXT m's QKV matmuls are
            # emitted, so the PE never waits on the norm/rope chain.
            pm, pqr, pkr = prev
            ptr = psum.tile([128, REP + 1, 128], BF16, tag="ps2", bufs=3, name="ptr")
            for h in range(REP):
                nc.tensor.transpose(ptr[:, h], pqr[:, h], ident)
            nc.tensor.transpose(ptr[:, REP], pkr, ident)
            nc.vector.tensor_copy(QT[:, :, pm * 128:(pm + 1) * 128],
                                  ptr[:, 0:REP])
            nc.vector.tensor_copy(KTr[:, pm * 128:(pm + 1) * 128],
                                  ptr[:, REP])

        # ---- Phase A: QKV + RMSNorm + RoPE + transpose ----
        prev = None
        for m in range(MT):
            xt, qtab, ktab = loaded0 if m == 0 else load_m(m)

            psq = psum.tile([128, QDIM], F32, tag="ps2", bufs=3, name="psq")
            pskv = psum.tile([128, 256], F32, tag="ps", name="pskv")
            for k in range(KT):
                nc.tensor.matmul(psq, xt[:, k], wq_sb[:, k],
                                 start=(k == 0), stop=(k == KT - 1))
            for k in range(KT):
                nc.tensor.matmul(pskv, xt[:, k], wkv_sb[:, k],
                                 start=(k == 0), stop=(k == KT - 1))

            if prev is not None:
                transpose_prev(prev)

            # RMSNorm q: one wide square (ACT) + per-head reduce (DVE)
            qn = wpool.tile([128, REP, HD], BF16, tag="qn")
            sq = wpool.tile([128, REP, HD], F32, tag="sq")
            nc.scalar.activation(out=sq, in_=psq, func=AF.Square)
            ssq4 = wpool.tile([128, REP], F32, tag="ssq4")
            nc.vector.reduce_sum(out=ssq4, in_=sq, axis=mybir.AxisListType.X)
            std4 = wpool.tile([128, REP], F32, tag="std4")
            nc.scalar.activation(out=std4, in_=ssq4, func=AF.Sqrt,
                                 bias=eps_sb, scale=1.0 / HD)
            r4 = wpool.tile([128, REP], F32, tag="r4")
            nc.vector.reciprocal(r4, std4)
            for h in range(REP):
                nc.vector.tensor_scalar_mul(qn[:, h],
                                            psq[:, h * HD:(h + 1) * HD],
                                            r4[:, h:h + 1])
            kn = wpool.tile([128, HD], BF16, tag="kn")
            sqk = wpool.tile([128, HD], F32, tag="sqk")
            ssqk = wpool.tile([128, 1], F32, tag="ssqk")
            nc.scalar.activation(out=sqk, in_=pskv[:, 0:HD],
                                 func=AF.Square, accum_out=ssqk)
            stdk = wpool.tile([128, 1], F32, tag="stdk")
            nc.scalar.activation(out=stdk, in_=ssqk, func=AF.Sqrt,
                                 bias=eps_sb, scale=1.0 / HD)
            rk = wpool.tile([128, 1], F32, tag="rk")
            nc.vector.reciprocal(rk, stdk)
            nc.vector.tensor_scalar_mul(kn, pskv[:, 0:HD], rk)

            # V: plain copy PSUM -> SBUF (bf16 cast)
            nc.scalar.activation(out=Vt[:, m], in_=pskv[:, HD:256], func=AF.Copy)

            # RoPE q (batched over heads; tables already head-replicated)
            qa = wpool.tile([128, REP, 64], BF16, tag="qa")
            qb2 = wpool.tile([128, REP, 64], BF16, tag="qb2")
            qr = wpool.tile([128, REP, HD], BF16, tag="qr")
            nc.vector.tensor_mul(qa, qn[:, :, 0:64], qtab[:, 0:4])
            nc.vector.tensor_mul(qb2, qn[:, :, 64:128], qtab[:, 4:8])
            nc.vector.tensor_sub(qr[:, :, 0:64], qa, qb2)
            qc = wpool.tile([128, REP, 64], BF16, tag="qc")
            qd = wpool.tile([128, REP, 64], BF16, tag="qd")
            nc.vector.tensor_mul(qc, qn[:, :, 0:64], qtab[:, 8:12])
            nc.vector.tensor_mul(qd, qn[:, :, 64:128], qtab[:, 12:16])
            nc.vector.tensor_add(qr[:, :, 64:128], qc, qd)

            # RoPE k
            ka = wpool.tile([128, 64], BF16, tag="ka")
            kb = wpool.tile([128, 64], BF16, tag="kb")
            kr = wpool.tile([128, HD], BF16, tag="kr")
            nc.vector.tensor_mul(ka, kn[:, 0:64], ktab[:, 0])
            nc.vector.tensor_mul(kb, kn[:, 64:128], ktab[:, 1])
            nc.vector.tensor_sub(kr[:, 0:64], ka, kb)
            kc = wpool.tile([128, 64], BF16, tag="kc")
            kd = wpool.tile([128, 64], BF16, tag="kd")
            nc.vector.tensor_mul(kc, kn[:, 0:64], ktab[:, 2])
            nc.vector.tensor_mul(kd, kn[:, 64:128], ktab[:, 3])
            nc.vector.tensor_add(kr[:, 64:128], kc, kd)

            prev = (m, qr, kr)
        # prev (m=15) transpose is deferred into the first attention group
        # so the PE doesn't stall on the last norm/rope chain.

        def attention(h, qb, defer=None):
            qs = slice(qb * 512, (qb + 1) * 512)
            po = psum.tile([128, 512], F32, tag="ps", name="po")
            pd = psum.tile([1, 512], F32, tag="ps", name="pd")
            NP = MT // 2
            pts = [None] * NP

            dsums = [None] * NP
            dquads = [None] * (NP // 2)
            docts = [None] * (NP // 4)

            def s_exp2(j):
                # kt pair (2j, 2j+1) into one 2-bank PSUM tile; single wide
                # exp on ACT amortizes the per-op overhead below PE pace.
                ps2 = psum.tile([128, 2, 512], F32, tag="ps2", bufs=3,
                                name="ps2")
                k0, k1 = 2 * j, 2 * j + 1
                nc.tensor.matmul(ps2[:, 0], KTr[:, k0 * 128:(k0 + 1) * 128],
                                 QT[:, h, qs], start=True, stop=True)
                if defer is not None and j == NP - 1:
                    transpose_prev(defer)
                nc.tensor.matmul(ps2[:, 1], KTr[:, k1 * 128:(k1 + 1) * 128],
                                 QT[:, h, qs], start=True, stop=True)
                pt = ppool.tile([128, 2, 512], BF16, tag="pt", name="pt")
                nc.scalar.activation(out=pt, in_=ps2, func=AF.Exp,
                                     scale=SCALE)
                pts[j] = pt
                # pair-sum on DVE halves the denominator matmuls on PE
                ds = dpool.tile([128, 512], BF16, tag="ds", name="ds")
                nc.vector.tensor_add(ds, pt[:, 0], pt[:, 1])
                dsums[j] = ds
                if j % 2 == 1:
                    # quad-sum: one denominator matmul per 4 kt
                    dq = dpool.tile([128, 512], BF16, tag="dq", name="dq")
                    nc.vector.tensor_add(dq, dsums[j - 1], ds)
                    dquads[j // 2] = dq
                if j % 4 == 3:
                    do_ = dpool.tile([128, 512], BF16, tag="do_", name="do_")
                    nc.vector.tensor_add(do_, dquads[j // 2 - 1],
                                         dquads[j // 2])
                    docts[j // 4] = do_

            def pvd2(j):
                for s in (0, 1):
                    k = 2 * j + s
                    nc.tensor.matmul(po, Vt[:, k], pts[j][:, s],
                                     start=(k == 0), stop=(k == MT - 1))
                if j % 4 == 3:
                    nc.tensor.matmul(pd, ones_sb, docts[j // 4],
                                     start=(j == 3), stop=(j == NP - 1))

            for j in range(NP):
                s_exp2(j)
                if j >= 1:
                    pvd2(j - 1)
            pvd2(NP - 1)
            rec = apool.tile([1, 512], F32, tag="rec", name="rec")
            nc.vector.reciprocal_approx_fast(out=rec, in_=pd)
            rb = apool.tile([128, 512], F32, tag="rb", name="rb")
            nc.gpsimd.partition_broadcast(rb, rec)
            nc.vector.tensor_mul(OT[:, h, qs], po, rb)

        def out_proj(mm):
            ms = slice(mm * 128, (mm + 1) * 128)
            for obp in range(2):
                # two output blocks share one 2-bank tile: 8 wait-free
                # matmuls (LDWs hide) + a single wide copy
                pp = psum.tile([128, 2, 512], F32, tag="ps2", bufs=3,
                               name="pp")
                for h in range(REP):
                    for s in (0, 1):
                        ob = 2 * obp + s
                        nc.tensor.matmul(pp[:, s], OT[:, h, ms],
                                         wo_sb[:, h, ob * 512:(ob + 1) * 512],
                                         start=(h == 0), stop=(h == REP - 1))
                osb = opool.tile([128, 1024], F32, tag="osb", name="osb")
                nc.scalar.activation(out=osb, in_=pp, func=AF.Copy)
                nc.sync.dma_start(
                    out[ms, obp * 1024:(obp + 1) * 1024], osb)

        # ---- Phase B (qb-outer) with phase C pipelined one qb behind ----
        for qb in range(NQB):
            for h in range(REP):
                attention(h, qb, defer=prev if (qb == 0 and h == 0) else None)
            if qb >= 1:
                for mm in range(4 * (qb - 1), 4 * qb):
                    out_proj(mm)
        for mm in range(4 * (NQB - 1), 4 * NQB):
            out_proj(mm)


def _build_nc():
    nc = bacc.Bacc("TRN2", target_bir_lowering=False, debug=False,
                   num_devices=NCORES)
    xmu = nc.dram_tensor("xmu", [MT, 128, KT, 128], BF16,
                         kind="ExternalInput").ap()
    wq = nc.dram_tensor("wq", [128, KT, QDIM], BF16,
                        kind="ExternalInput").ap()
    wkv = nc.dram_tensor("wkv", [128, KT, 256], BF16,
                         kind="ExternalInput").ap()
    wo = nc.dram_tensor("wo", [128, REP, D], BF16, kind="ExternalInput").ap()
    tabs = nc.dram_tensor("tabs", [MT, 128, 20, 64], BF16,
                          kind="ExternalInput").ap()
    out = nc.dram_tensor("out", [N, D], F32, kind="ExternalOutput").ap()
    with tile.TileContext(nc) as tc:
        _kernel_body(tc, xmu, wq, wkv, wo, tabs, out)
    nc.compile()
    return nc


_PERM = np.concatenate([np.arange(0, HD, 2), np.arange(1, HD, 2)])


def _bf16(a):
    return np.ascontiguousarray(a).astype(ml_dtypes.bfloat16)


def _prep_shared(inputs):
    """Host prep that doesn't depend on the core: tables + per-(b,g) arrays."""
    x = np.asarray(inputs["x"], np.float32)
    mu = np.asarray(inputs["mu_prev"], np.float32)
    cos = np.asarray(inputs["cos"], np.float32)
    sin = np.asarray(inputs["sin"], np.float32)
    qnw = np.asarray(inputs["q_norm_w"], np.float32)
    knw = np.asarray(inputs["k_norm_w"], np.float32)
    wq = np.asarray(inputs["wq"], np.float32)
    wk = np.asarray(inputs["wk"], np.float32)
    wv = np.asarray(inputs["wv"], np.float32)
    mqw = np.asarray(inputs["mu_q_w"], np.float32)
    mkw = np.asarray(inputs["mu_k_w"], np.float32)
    mvw = np.asarray(inputs["mu_v_w"], np.float32)
    wo = np.asarray(inputs["wo"], np.float32)

    # RoPE tables with norm weights folded in (permuted even/odd space):
    # out1 = t1*C1 - t2*S2 ; out2 = t1*S1 + t2*C2
    we, wo_ = qnw[0::2], qnw[1::2]
    qparts = [cos * we, sin * wo_, sin * we, cos * wo_]
    we_k, wo_k = knw[0::2], knw[1::2]
    kparts = [cos * we_k, sin * wo_k, sin * we_k, cos * wo_k]
    tab_list = [qparts[j] for j in range(4) for _ in range(REP)] + kparts
    tabs = np.stack(tab_list, axis=1)                # [N, 20, 64]
    tabs_arr = _bf16(tabs.reshape(MT, 128, 20, 64))

    # Per-batch xmu, pre-tiled [m, f, kt, t]
    xmu_arrs = []
    for b in range(B):
        xm = np.concatenate([x[b], mu[b]], axis=1)   # [N, 4096]
        xm = xm.reshape(MT, 128, KT, 128).transpose(0, 3, 2, 1)
        xmu_arrs.append(_bf16(xm))

    # Per-group weights
    wq_arrs, wkv_arrs, wo_arrs = [], [], []
    for g in range(KVH):
        qs = slice(g * QDIM, (g + 1) * QDIM)
        kvs = slice(g * HD, (g + 1) * HD)
        perm_q = np.concatenate([hh * HD + _PERM for hh in range(REP)])
        Wq = np.concatenate([wq[qs], mqw[qs]], axis=1)[perm_q]   # [512, 4096]
        Wk = np.concatenate([wk[kvs], mkw[kvs]], axis=1)[_PERM]  # [128, 4096]
        Wv = np.concatenate([wv[kvs], mvw[kvs]], axis=1)         # [128, 4096]
        Wg = np.concatenate([Wq, Wk, Wv], axis=0)                # [768, 4096]
        wg_t = Wg.T.reshape(KT, 128, 768).transpose(1, 0, 2)
        wq_arrs.append(_bf16(wg_t[:, :, 0:QDIM]))
        wkv_arrs.append(_bf16(wg_t[:, :, QDIM:768]))
        wo_g = wo[:, g * QDIM:(g + 1) * QDIM].T                  # [512, D]
        wo_arrs.append(_bf16(
            wo_g.reshape(REP, HD, D).transpose(1, 0, 2)))

    in_maps = []
    for c in range(NCORES):
        b, g = divmod(c, KVH)
        in_maps.append({
            "xmu": xmu_arrs[b],
            "wq": wq_arrs[g],
            "wkv": wkv_arrs[g],
            "wo": wo_arrs[g],
            "tabs": tabs_arr,
        })
    return in_maps


def _install_ntff_hook():
    try:
        import antenv.axon_hooks as m
        if m.get_axon_ntff_profile_hook() is not None:
            return True
    except ImportError:
        import antenv
        m = types.ModuleType("antenv.axon_hooks")
        m._hook = None
        m.set_axon_ntff_profile_hook = lambda h: setattr(m, "_hook", h)
        m.get_axon_ntff_profile_hook = lambda: m._hook
        sys.modules["antenv.axon_hooks"] = m
        antenv.axon_hooks = m
    try:
        from trn_agent_boot.trn_boot import _ntff_profile_via_ctypes
        m.set_axon_ntff_profile_hook(
            _ntff_profile_via_ctypes("/opt/axon/libaxon_pjrt.so"))
    except Exception:
        return False
    return m.get_axon_ntff_profile_hook() is not None


def run(inputs, trace=False, tmpdir=None):
    """Returns (output [B,N,D] f32, BassKernelResults)."""
    global _nc_cache
    if trace:
        _install_ntff_hook()
    if _nc_cache is None:
        _nc_cache = _build_nc()
    in_maps = _prep_shared(inputs)
    res = run_bass_kernel_spmd(_nc_cache, in_maps,
                               core_ids=list(range(NCORES)),
                               trace=trace, tmpdir=tmpdir)
    parts = np.stack([np.asarray(res.results[c]["out"], np.float32)
                      for c in range(NCORES)])
    outv = parts.reshape(B, KVH, N, D).sum(axis=1).astype(np.float32)
    return outv, res


def kernel(**inputs):
    outv, _ = run(inputs, trace=False)
    return outv
